# revision 19
# baseline (speedup 1.0000x reference)
"""MLA attention kernel for 8 Trainium2 NeuronCores.

Sharding: data-parallel over batch (B=2) x tensor-parallel over heads
(16 heads -> 4 per core). Each core computes full-sequence causal
attention for its 4 heads and a *partial* output projection
(y_part = out_heads @ W_o[head_rows]); the host sums the 4 partials per
batch element.

Dataflow is feature-major ("transposed activations") so no on-chip
transposes are needed anywhere:
  x^T (host-pretransposed, bf16) -> lat^T = W_kv^T x^T -> K^T = W_k^T lat^T
  Q^T = W_q^T x^T,  V = lat^T.T W_v  (token-major, with a ones column
  appended per head so the P@V matmul also yields softmax denominators)
  S^T = K^T.T Q^T per (head, 128-key-block, 512-query-chunk), exp on ACT
  (no max subtraction: |S*scale| <= ~8 for this problem), causal via
  static block skipping + per-block masks on the 4 diagonal blocks,
  out^T accumulated in PSUM over key blocks, normalized by 1/l, then
  y_part^T ... = out^T rows as lhsT for the W_o projection.
"""

import sys

sys.path.insert(0, "/opt/trn_rl_repo")

import numpy as np
import ml_dtypes

BF = ml_dtypes.bfloat16

B, T, D = 2, 4096, 1024
H, DH, LAT = 16, 64, 32
H_LOC = 4              # heads per core
HW = H_LOC * DH        # 256, per-core head width
QC = 512               # query chunk
NQC = T // QC          # 8
KB = 128               # key block
NKB = T // KB          # 32
SCALE = 1.0 / np.sqrt(np.float32(DH))  # 0.125
N_CORES = 8

_prog_cache = {}


def _build_program():
    import concourse.bacc as bacc
    import concourse.tile as tile
    import concourse.mybir as mybir

    f32 = mybir.dt.float32
    bf16 = mybir.dt.bfloat16
    Exp = mybir.ActivationFunctionType.Exp

    nc = bacc.Bacc(None, target_bir_lowering=False)

    xT = nc.dram_tensor("xT", [D, T], bf16, kind="ExternalInput")
    wq = nc.dram_tensor("wq", [D, HW], bf16, kind="ExternalInput")
    wkv = nc.dram_tensor("wkv", [D, LAT], bf16, kind="ExternalInput")
    wk = nc.dram_tensor("wk", [128, HW], bf16, kind="ExternalInput")   # zero-padded 32->128
    wv = nc.dram_tensor("wv", [128, HW], bf16, kind="ExternalInput")   # zero-padded 32->128
    wo = nc.dram_tensor("wo", [HW, D], bf16, kind="ExternalInput")
    msk = nc.dram_tensor("msk", [128, 4, 2 * QC], bf16, kind="ExternalInput")
    y = nc.dram_tensor("y", [T, D], f32, kind="ExternalOutput")

    DSUB = D // 128  # 8

    with tile.TileContext(nc) as tc:
        with (
            tc.tile_pool(name="singles", bufs=1) as singles,
            tc.tile_pool(name="work", bufs=6) as work,
            tc.tile_pool(name="pT", bufs=8) as ppool,
            tc.tile_pool(name="ot", bufs=6) as opool,
            tc.tile_pool(name="ysb", bufs=3) as ypool,
            tc.tile_pool(name="lv", bufs=8) as lpool,
            tc.tile_pool(name="ps_s", bufs=2, space="PSUM") as ps_s,
            tc.tile_pool(name="ps_o", bufs=1, space="PSUM") as ps_o,
            tc.tile_pool(name="ps_m", bufs=2, space="PSUM") as ps_m,
        ):
            # ---- persistent SBUF tensors -------------------------------
            # small weights needed by the first chunk go first; the rest are
            # DMA'd after the first x-chunk so chunk 0 isn't starved
            wkv_sb = singles.tile([128, DSUB, LAT], bf16)
            nc.sync.dma_start(wkv_sb, wkv.rearrange("(o p) m -> p o m", p=128))
            wk_sb = singles.tile([128, HW], bf16)
            nc.sync.dma_start(wk_sb, wk[:, :])
            wv_sb = singles.tile([128, HW], bf16)
            nc.sync.dma_start(wv_sb, wv[:, :])
            wq_sb = singles.tile([128, DSUB, HW], bf16)
            wo_sb = singles.tile([128, 2, D], bf16)
            msk_sb = singles.tile([128, 4, 2 * QC], bf16)

            def emit_late_weight_loads():
                nc.sync.dma_start(wq_sb, wq.rearrange("(o p) m -> p o m", p=128))
                nc.sync.dma_start(wo_sb, wo.rearrange("(r p) d -> p r d", p=128))
                nc.sync.dma_start(msk_sb, msk[:, :, :])

            ones_w = DH + 1  # 65
            # per-chunk persistent tiles: breaks write-after-read false deps
            # between chunk tc's projections and chunk tc-1's attention
            kt_c = []
            v1_c = []
            for c in range(NQC):
                kt_1 = singles.tile([128, 2, QC], bf16, name=f"kt_{c}")
                kt_c.append(kt_1)
                v1_1 = singles.tile(
                    [128, QC // KB, H_LOC * ones_w], bf16, name=f"v1_{c}"
                )
                nc.vector.memset(v1_1, 1.0)  # ones columns survive V copies
                v1_c.append(v1_1)

            qt_tiles = {}

            xt_tiles = {}

            def emit_xt_dma(tcidx):
                t0 = tcidx * QC
                xt = work.tile([128, DSUB, QC], bf16, tag="xt", name="xt")
                xTv = xT.rearrange("(o p) t -> p o t", p=128)
                nc.sync.dma_start(xt[:, 0:2], xTv[:, 0:2, t0 : t0 + QC])
                nc.sync.dma_start(xt[:, 2:DSUB], xTv[:, 2:DSUB, t0 : t0 + QC])
                xt_tiles[tcidx] = xt

            def emit_phase0_a(tcidx):
                if tcidx not in xt_tiles:
                    emit_xt_dma(tcidx)
                xt = xt_tiles.pop(tcidx)
                # lat^T chunk (local tile; rows 32.. zeroed for the padded
                # contractions in the K/V projections)
                latT = work.tile([128, QC], bf16, tag="latT", name="latT")
                nc.vector.memset(latT, 0.0)
                lt_ps = ps_m.tile([128, QC], f32, tag="m", name="lt_ps")
                for o in range(DSUB):
                    nc.tensor.matmul(
                        lt_ps[0:LAT, 0:QC],
                        lhsT=wkv_sb[:, o],
                        rhs=xt[:, o],
                        start=(o == 0),
                        stop=(o == DSUB - 1),
                    )
                nc.vector.tensor_copy(latT[0:LAT, :], lt_ps[0:LAT, 0:QC])
                # K^T chunk (2 head pairs)
                for p in range(2):
                    ktp = ps_m.tile([128, QC], f32, tag="m", name="ktp")
                    nc.tensor.matmul(
                        ktp[:, 0:QC],
                        lhsT=wk_sb[:, 128 * p : 128 * p + 128],
                        rhs=latT,
                        start=True,
                        stop=True,
                    )
                    nc.vector.tensor_copy(kt_c[tcidx][:, p, :], ktp[:, 0:QC])
                qt_tiles[tcidx] = (xt, latT)

            def emit_phase0_b(tcidx):
                xt, latT = qt_tiles[tcidx]
                # V chunk (4 key blocks of 128)
                for s in range(QC // KB):
                    vps = ps_m.tile([128, QC], f32, tag="m", name="vps")
                    nc.tensor.matmul(
                        vps[:, 0:HW],
                        lhsT=latT[:, s * KB : s * KB + KB],
                        rhs=wv_sb,
                        start=True,
                        stop=True,
                    )
                    nc.vector.tensor_copy(
                        v1_c[tcidx][:, s].rearrange("p (h e) -> p h e", e=ones_w)[
                            :, :, 0:DH
                        ],
                        vps[:, 0:HW].rearrange("p (h d) -> p h d", d=DH),
                    )
                # Q^T chunk (2 head pairs)
                qt = work.tile([128, 2, QC], bf16, tag="qt", name="qt")
                for p in range(2):
                    qtp = ps_m.tile([128, QC], f32, tag="m", name="qtp")
                    for o in range(DSUB):
                        nc.tensor.matmul(
                            qtp[:, 0:QC],
                            lhsT=wq_sb[:, o, 128 * p : 128 * p + 128],
                            rhs=xt[:, o],
                            start=(o == 0),
                            stop=(o == DSUB - 1),
                        )
                    nc.vector.tensor_copy(qt[:, p, :], qtp[:, 0:QC])
                qt_tiles[tcidx] = qt

            def emit_phase0(tcidx):
                emit_phase0_a(tcidx)
                emit_phase0_b(tcidx)

            def emit_attention_pair(qc, pair, ots):
                qt = qt_tiles[qc]
                nkb = (qc + 1) * (QC // KB)
                if True:
                    o_ps = []
                    for h01 in range(2):
                        o_ps_h = ps_o.tile(
                            [128, QC], f32, tag=f"o{h01}", name=f"o_ps_{h01}"
                        )
                        o_ps.append(o_ps_h)
                    for kb in range(nkb):
                        # diagonal blocks: queries < j*128 are fully masked;
                        # restrict S/exp/mask/PV to the live column subrange
                        j = kb - (nkb - 4)
                        qlo = j * KB if j > 0 else 0
                        qn = QC - qlo
                        s_ps = ps_s.tile([128, 2 * QC], f32, tag="s", name="s_ps")
                        for h01 in range(2):
                            nc.tensor.matmul(
                                s_ps[:, h01 * QC + qlo : h01 * QC + QC],
                                lhsT=kt_c[kb // 4][
                                    64 * h01 : 64 * h01 + 64,
                                    pair,
                                    (kb % 4) * KB : (kb % 4) * KB + KB,
                                ],
                                rhs=qt[64 * h01 : 64 * h01 + 64, pair, qlo:QC],
                                start=True,
                                stop=True,
                            )
                        pT = ppool.tile([128, 2 * QC], bf16, tag="pT", name="pT")
                        s_v = s_ps.rearrange("p (h q) -> p h q", h=2)
                        p_v = pT.rearrange("p (h q) -> p h q", h=2)
                        nc.scalar.activation(
                            p_v[:, :, qlo:QC],
                            s_v[:, :, qlo:QC],
                            Exp,
                            scale=float(SCALE),
                        )
                        if j >= 0:
                            nc.vector.tensor_mul(
                                p_v[:, :, qlo:QC],
                                p_v[:, :, qlo:QC],
                                msk_sb[:, j].rearrange("p (h q) -> p h q", h=2)[
                                    :, :, qlo:QC
                                ],
                            )
                        for h01 in range(2):
                            h = 2 * pair + h01
                            nc.tensor.matmul(
                                o_ps[h01][0 : ones_w, qlo:QC],
                                lhsT=v1_c[kb // 4][
                                    :, kb % 4, ones_w * h : ones_w * h + ones_w
                                ],
                                rhs=pT[:, h01 * QC + qlo : h01 * QC + QC],
                                start=(kb == 0),
                                stop=(kb == nkb - 1),
                            )
                    # normalize: ot[64*h01 + d, q] = o_ps[h01][d, q] / l
                    ot = opool.tile([128, QC], bf16, tag="ot", name="ot")
                    for h01 in range(2):
                        linv = lpool.tile([1, QC], f32, tag="linv", name="linv")
                        nc.vector.reciprocal(linv, o_ps[h01][DH : DH + 1, :])
                        lb = lpool.tile([64, QC], f32, tag="lb", name="lb")
                        nc.gpsimd.partition_broadcast(lb, linv)
                        nc.vector.tensor_mul(
                            ot[64 * h01 : 64 * h01 + 64, :],
                            o_ps[h01][0:DH, :],
                            lb,
                        )
                    ots.append(ot)

            def emit_y(qc, ots):
                qt_tiles.pop(qc, None)
                q0 = qc * QC
                # ---- output projection for this chunk ------------------
                for tt in range(QC // 128):
                    for nh in range(2):
                        y_ps = ps_m.tile([128, QC], f32, tag="m", name="y_ps")
                        for p2 in range(2):
                            nc.tensor.matmul(
                                y_ps,
                                lhsT=ots[p2][:, tt * 128 : tt * 128 + 128],
                                rhs=wo_sb[:, p2, nh * QC : nh * QC + QC],
                                start=(p2 == 0),
                                stop=(p2 == 1),
                            )
                        y_sb = ypool.tile([128, QC], f32, tag="ysb", name="y_sb")
                        nc.vector.tensor_copy(y_sb, y_ps)
                        nc.sync.dma_start(
                            y[q0 + tt * 128 : q0 + tt * 128 + 128,
                              nh * QC : nh * QC + QC],
                            y_sb,
                        )

            # software pipeline: phase0 of chunk tc+1 is emitted between the
            # two attention pairs of chunk tc, so per-engine in-order streams
            # overlap projections with the ACT-bound attention
            emit_phase0_a(0)
            emit_late_weight_loads()
            emit_phase0_b(0)
            ots_all = {}
            emit_xt_dma(1)
            emit_xt_dma(2)
            emit_phase0_a(1)
            for tcidx in range(NQC):
                if tcidx + 3 < NQC:
                    emit_xt_dma(tcidx + 3)
                ots = []
                ots_all[tcidx] = ots
                emit_attention_pair(tcidx, 0, ots)
                if tcidx - 1 in ots_all:
                    emit_y(tcidx - 1, ots_all.pop(tcidx - 1))
                if tcidx + 1 < NQC:
                    emit_phase0_b(tcidx + 1)
                emit_attention_pair(tcidx, 1, ots)
                if tcidx + 2 < NQC:
                    emit_phase0_a(tcidx + 2)
            emit_y(NQC - 1, ots_all.pop(NQC - 1))

    nc.finalize()
    return nc


def _make_masks():
    # msk[r, j, q (+QC dup)] = 1.0 if j*128 + r <= q else 0
    r = np.arange(128)[:, None, None]
    jj = np.arange(4)[None, :, None]
    q = np.arange(QC)[None, None, :]
    m = ((jj * KB + r) <= q).astype(BF)
    return np.concatenate([m, m], axis=2)  # duplicate for the head pair


def kernel(x, W_kv, W_k, W_v, W_q, W_o):
    import os

    # the axon NTFF profiling hook is unavailable in this environment;
    # force the plain execute path
    os.environ["BASS_NEVER_TRACE"] = "1"
    from concourse.bass_utils import run_bass_kernel_spmd

    x = np.asarray(x)
    W_kv, W_k, W_v, W_q, W_o = (
        np.asarray(W_kv),
        np.asarray(W_k),
        np.asarray(W_v),
        np.asarray(W_q),
        np.asarray(W_o),
    )

    if "nc" not in _prog_cache:
        _prog_cache["nc"] = _build_program()
    nc = _prog_cache["nc"]

    msk = _make_masks()
    in_maps = []
    for c in range(N_CORES):
        bidx, g = c // 4, c % 4
        wk_pad = np.zeros((128, HW), dtype=BF)
        wk_pad[0:LAT] = W_k[:, HW * g : HW * g + HW].astype(BF)
        wv_pad = np.zeros((128, HW), dtype=BF)
        wv_pad[0:LAT] = W_v[:, HW * g : HW * g + HW].astype(BF)
        in_maps.append(
            {
                "xT": np.ascontiguousarray(x[bidx].T).astype(BF),
                "wq": np.ascontiguousarray(W_q[:, HW * g : HW * g + HW]).astype(BF),
                "wkv": W_kv.astype(BF),
                "wk": wk_pad,
                "wv": wv_pad,
                "wo": np.ascontiguousarray(W_o[HW * g : HW * g + HW, :]).astype(BF),
                "msk": msk,
            }
        )

    res = run_bass_kernel_spmd(nc, in_maps, core_ids=list(range(N_CORES)))
    _prog_cache["last_res"] = res

    y = np.zeros((B, T, D), np.float32)
    for c in range(N_CORES):
        y[c // 4] += res.results[c]["y"]
    return y


# revision 29
# speedup vs baseline: 1.0080x; 1.0080x over previous
"""MLA attention kernel for 8 Trainium2 NeuronCores.

Sharding: data-parallel over batch (B=2) x tensor-parallel over heads
(16 heads -> 4 per core). Each core computes full-sequence causal
attention for its 4 heads and a *partial* output projection
(y_part = out_heads @ W_o[head_rows]); the host sums the 4 partials per
batch element.

Dataflow is feature-major ("transposed activations") so no on-chip
transposes are needed anywhere:
  x^T (host-pretransposed, bf16) -> lat^T = W_kv^T x^T -> K^T = W_k^T lat^T
  Q^T = W_q^T x^T,  V = lat^T.T W_v  (token-major, with a ones column
  appended per head so the P@V matmul also yields softmax denominators)
  S^T = K^T.T Q^T per (head, 128-key-block, 512-query-chunk), exp on ACT
  (no max subtraction: |S*scale| <= ~8 for this problem), causal via
  static block skipping + per-block masks on the 4 diagonal blocks,
  out^T accumulated in PSUM over key blocks, normalized by 1/l, then
  y_part^T ... = out^T rows as lhsT for the W_o projection.
"""

import sys

sys.path.insert(0, "/opt/trn_rl_repo")

import numpy as np
import ml_dtypes

BF = ml_dtypes.bfloat16

B, T, D = 2, 4096, 1024
H, DH, LAT = 16, 64, 32
H_LOC = 4              # heads per core
HW = H_LOC * DH        # 256, per-core head width
QC = 512               # query chunk
NQC = T // QC          # 8
KB = 128               # key block
NKB = T // KB          # 32
SCALE = 1.0 / np.sqrt(np.float32(DH))  # 0.125
N_CORES = 8

_prog_cache = {}


def _build_program():
    import concourse.bacc as bacc
    import concourse.tile as tile
    import concourse.mybir as mybir

    f32 = mybir.dt.float32
    bf16 = mybir.dt.bfloat16
    Exp = mybir.ActivationFunctionType.Exp

    nc = bacc.Bacc(None, target_bir_lowering=False)

    xT = nc.dram_tensor("xT", [D, T], bf16, kind="ExternalInput")
    wq = nc.dram_tensor("wq", [D, HW], bf16, kind="ExternalInput")
    wkv = nc.dram_tensor("wkv", [D, LAT], bf16, kind="ExternalInput")
    wk = nc.dram_tensor("wk", [128, HW], bf16, kind="ExternalInput")   # zero-padded 32->128
    wv = nc.dram_tensor("wv", [128, HW], bf16, kind="ExternalInput")   # zero-padded 32->128
    wo = nc.dram_tensor("wo", [HW, D], bf16, kind="ExternalInput")
    msk = nc.dram_tensor("msk", [128, 4, 2 * QC], bf16, kind="ExternalInput")
    y = nc.dram_tensor("y", [T, D], f32, kind="ExternalOutput")

    DSUB = D // 128  # 8

    with tile.TileContext(nc) as tc:
        with (
            tc.tile_pool(name="singles", bufs=1) as singles,
            tc.tile_pool(name="work", bufs=6) as work,
            tc.tile_pool(name="pT", bufs=8) as ppool,
            tc.tile_pool(name="ot", bufs=6) as opool,
            tc.tile_pool(name="ysb", bufs=3) as ypool,
            tc.tile_pool(name="lv", bufs=4) as lpool,
            tc.tile_pool(name="ps_s", bufs=2, space="PSUM") as ps_s,
            tc.tile_pool(name="ps_o", bufs=1, space="PSUM") as ps_o,
            tc.tile_pool(name="ps_m", bufs=2, space="PSUM") as ps_m,
        ):
            # ---- persistent SBUF tensors -------------------------------
            # small weights needed by the first chunk go first; the rest are
            # DMA'd after the first x-chunk so chunk 0 isn't starved
            wkv_sb = singles.tile([128, DSUB, LAT], bf16)
            nc.sync.dma_start(wkv_sb, wkv.rearrange("(o p) m -> p o m", p=128))
            wk_sb = singles.tile([128, HW], bf16)
            nc.sync.dma_start(wk_sb, wk[:, :])
            wv_sb = singles.tile([128, HW], bf16)
            nc.sync.dma_start(wv_sb, wv[:, :])
            wq_sb = singles.tile([128, DSUB, HW], bf16)
            wo_sb = singles.tile([128, 2, D], bf16)
            msk_sb = singles.tile([128, 4, 2 * QC], bf16)

            def emit_late_weight_loads():
                nc.sync.dma_start(wq_sb, wq.rearrange("(o p) m -> p o m", p=128))
                nc.sync.dma_start(wo_sb, wo.rearrange("(r p) d -> p r d", p=128))
                nc.sync.dma_start(msk_sb, msk[:, :, :])

            ones_w = DH + 1  # 65
            # per-chunk persistent tiles: breaks write-after-read false deps
            # between chunk tc's projections and chunk tc-1's attention
            kt_c = []
            v1_c = []
            for c in range(NQC):
                kt_1 = singles.tile([128, 2, QC], bf16, name=f"kt_{c}")
                kt_c.append(kt_1)
                v1_1 = singles.tile(
                    [128, QC // KB, H_LOC * ones_w], bf16, name=f"v1_{c}"
                )
                nc.vector.memset(v1_1, 1.0)  # ones columns survive V copies
                v1_c.append(v1_1)

            # warm the ACT exp table-set during the initial weight DMAs so
            # the first real exp doesn't pay the ~2.7us table load
            warm = lpool.tile([1, 1], f32, tag="warm", name="warm")
            nc.scalar.activation(warm, v1_c[0][0:1, 0:1, 0:1], Exp)

            qt_tiles = {}

            xt_tiles = {}

            def emit_xt_dma(tcidx):
                t0 = tcidx * QC
                xt = work.tile([128, DSUB, QC], bf16, tag="xt", name="xt")
                xTv = xT.rearrange("(o p) t -> p o t", p=128)
                nc.sync.dma_start(xt[:, 0:2], xTv[:, 0:2, t0 : t0 + QC])
                nc.sync.dma_start(xt[:, 2:DSUB], xTv[:, 2:DSUB, t0 : t0 + QC])
                xt_tiles[tcidx] = xt

            def emit_phase0_a(tcidx):
                if tcidx not in xt_tiles:
                    emit_xt_dma(tcidx)
                xt = xt_tiles.pop(tcidx)
                # lat^T chunk (local tile; rows 32.. zeroed for the padded
                # contractions in the K/V projections)
                latT = work.tile([128, QC], bf16, tag="latT", name="latT")
                nc.vector.memset(latT, 0.0)
                lt_ps = ps_m.tile([128, QC], f32, tag="m", name="lt_ps")
                for o in range(DSUB):
                    nc.tensor.matmul(
                        lt_ps[0:LAT, 0:QC],
                        lhsT=wkv_sb[:, o],
                        rhs=xt[:, o],
                        start=(o == 0),
                        stop=(o == DSUB - 1),
                    )
                nc.vector.tensor_copy(latT[0:LAT, :], lt_ps[0:LAT, 0:QC])
                # K^T chunk (2 head pairs)
                for p in range(2):
                    ktp = ps_m.tile([128, QC], f32, tag="m", name="ktp")
                    nc.tensor.matmul(
                        ktp[:, 0:QC],
                        lhsT=wk_sb[:, 128 * p : 128 * p + 128],
                        rhs=latT,
                        start=True,
                        stop=True,
                    )
                    nc.vector.tensor_copy(kt_c[tcidx][:, p, :], ktp[:, 0:QC])
                qt_tiles[tcidx] = (xt, latT)

            def emit_phase0_b(tcidx):
                xt, latT = qt_tiles[tcidx]
                # V chunk (4 key blocks of 128)
                for s in range(QC // KB):
                    vps = ps_m.tile([128, QC], f32, tag="m", name="vps")
                    nc.tensor.matmul(
                        vps[:, 0:HW],
                        lhsT=latT[:, s * KB : s * KB + KB],
                        rhs=wv_sb,
                        start=True,
                        stop=True,
                    )
                    nc.vector.tensor_copy(
                        v1_c[tcidx][:, s].rearrange("p (h e) -> p h e", e=ones_w)[
                            :, :, 0:DH
                        ],
                        vps[:, 0:HW].rearrange("p (h d) -> p h d", d=DH),
                    )
                # Q^T chunk (2 head pairs)
                qt = work.tile([128, 2, QC], bf16, tag="qt", name="qt")
                for p in range(2):
                    qtp = ps_m.tile([128, QC], f32, tag="m", name="qtp")
                    for o in range(DSUB):
                        nc.tensor.matmul(
                            qtp[:, 0:QC],
                            lhsT=wq_sb[:, o, 128 * p : 128 * p + 128],
                            rhs=xt[:, o],
                            start=(o == 0),
                            stop=(o == DSUB - 1),
                        )
                    nc.vector.tensor_copy(qt[:, p, :], qtp[:, 0:QC])
                qt_tiles[tcidx] = qt

            def emit_phase0(tcidx):
                emit_phase0_a(tcidx)
                emit_phase0_b(tcidx)

            def emit_attention_pair(qc, pair, ots):
                qt = qt_tiles[qc]
                nkb = (qc + 1) * (QC // KB)
                if True:
                    o_ps = []
                    for h01 in range(2):
                        o_ps_h = ps_o.tile(
                            [128, QC], f32, tag=f"o{h01}", name=f"o_ps_{h01}"
                        )
                        o_ps.append(o_ps_h)
                    for kb in range(nkb):
                        # diagonal blocks: queries < j*128 are fully masked;
                        # restrict S/exp/mask/PV to the live column subrange
                        j = kb - (nkb - 4)
                        qlo = j * KB if j > 0 else 0
                        qn = QC - qlo
                        s_ps = ps_s.tile([128, 2 * QC], f32, tag="s", name="s_ps")
                        for h01 in range(2):
                            nc.tensor.matmul(
                                s_ps[:, h01 * QC + qlo : h01 * QC + QC],
                                lhsT=kt_c[kb // 4][
                                    64 * h01 : 64 * h01 + 64,
                                    pair,
                                    (kb % 4) * KB : (kb % 4) * KB + KB,
                                ],
                                rhs=qt[64 * h01 : 64 * h01 + 64, pair, qlo:QC],
                                start=True,
                                stop=True,
                            )
                        pT = ppool.tile([128, 2 * QC], bf16, tag="pT", name="pT")
                        s_v = s_ps.rearrange("p (h q) -> p h q", h=2)
                        p_v = pT.rearrange("p (h q) -> p h q", h=2)
                        nc.scalar.activation(
                            p_v[:, :, qlo:QC],
                            s_v[:, :, qlo:QC],
                            Exp,
                            scale=float(SCALE),
                        )
                        if j >= 0:
                            nc.vector.tensor_mul(
                                p_v[:, :, qlo:QC],
                                p_v[:, :, qlo:QC],
                                msk_sb[:, j].rearrange("p (h q) -> p h q", h=2)[
                                    :, :, qlo:QC
                                ],
                            )
                        for h01 in range(2):
                            h = 2 * pair + h01
                            nc.tensor.matmul(
                                o_ps[h01][0 : ones_w, qlo:QC],
                                lhsT=v1_c[kb // 4][
                                    :, kb % 4, ones_w * h : ones_w * h + ones_w
                                ],
                                rhs=pT[:, h01 * QC + qlo : h01 * QC + QC],
                                start=(kb == 0),
                                stop=(kb == nkb - 1),
                            )
                    # copy accumulators to SBUF right away so the PSUM
                    # banks free for the next pair, then normalize from SBUF
                    ot = opool.tile([128, QC], bf16, tag="ot", name="ot")
                    osb = []
                    for h01 in range(2):
                        osb_h = lpool.tile(
                            [ones_w, QC], f32, tag=f"osb{h01}", name="osb_h"
                        )
                        nc.vector.tensor_copy(osb_h, o_ps[h01][0:ones_w, :])
                        osb.append(osb_h)
                    for h01 in range(2):
                        linv = lpool.tile([1, QC], f32, tag="linv", name="linv")
                        nc.vector.reciprocal(linv, osb[h01][DH : DH + 1, :])
                        lb = lpool.tile([64, QC], f32, tag="lb", name="lb")
                        nc.gpsimd.partition_broadcast(lb, linv)
                        nc.vector.tensor_mul(
                            ot[64 * h01 : 64 * h01 + 64, :],
                            osb[h01][0:DH, :],
                            lb,
                        )
                    ots.append(ot)

            def emit_y(qc, ots):
                qt_tiles.pop(qc, None)
                q0 = qc * QC
                # ---- output projection for this chunk ------------------
                for tt in range(QC // 128):
                    for nh in range(2):
                        y_ps = ps_m.tile([128, QC], f32, tag="m", name="y_ps")
                        for p2 in range(2):
                            nc.tensor.matmul(
                                y_ps,
                                lhsT=ots[p2][:, tt * 128 : tt * 128 + 128],
                                rhs=wo_sb[:, p2, nh * QC : nh * QC + QC],
                                start=(p2 == 0),
                                stop=(p2 == 1),
                            )
                        y_sb = ypool.tile([128, QC], f32, tag="ysb", name="y_sb")
                        nc.vector.tensor_copy(y_sb, y_ps)
                        nc.sync.dma_start(
                            y[q0 + tt * 128 : q0 + tt * 128 + 128,
                              nh * QC : nh * QC + QC],
                            y_sb,
                        )

            # software pipeline: phase0 of chunk tc+1 is emitted between the
            # two attention pairs of chunk tc, so per-engine in-order streams
            # overlap projections with the ACT-bound attention
            emit_phase0_a(0)
            emit_late_weight_loads()
            emit_phase0_b(0)
            ots_all = {}
            emit_xt_dma(1)
            emit_xt_dma(2)
            emit_phase0_a(1)
            for tcidx in range(NQC):
                if tcidx + 3 < NQC:
                    emit_xt_dma(tcidx + 3)
                ots = []
                ots_all[tcidx] = ots
                emit_attention_pair(tcidx, 0, ots)
                if tcidx - 1 in ots_all:
                    emit_y(tcidx - 1, ots_all.pop(tcidx - 1))
                if tcidx + 1 < NQC:
                    emit_phase0_b(tcidx + 1)
                emit_attention_pair(tcidx, 1, ots)
                if tcidx + 2 < NQC:
                    emit_phase0_a(tcidx + 2)
            emit_y(NQC - 1, ots_all.pop(NQC - 1))

    nc.finalize()
    return nc


def _make_masks():
    # msk[r, j, q (+QC dup)] = 1.0 if j*128 + r <= q else 0
    r = np.arange(128)[:, None, None]
    jj = np.arange(4)[None, :, None]
    q = np.arange(QC)[None, None, :]
    m = ((jj * KB + r) <= q).astype(BF)
    return np.concatenate([m, m], axis=2)  # duplicate for the head pair


def kernel(x, W_kv, W_k, W_v, W_q, W_o):
    import os

    # the axon NTFF profiling hook is unavailable in this environment;
    # force the plain execute path
    os.environ["BASS_NEVER_TRACE"] = "1"
    from concourse.bass_utils import run_bass_kernel_spmd

    x = np.asarray(x)
    W_kv, W_k, W_v, W_q, W_o = (
        np.asarray(W_kv),
        np.asarray(W_k),
        np.asarray(W_v),
        np.asarray(W_q),
        np.asarray(W_o),
    )

    if "nc" not in _prog_cache:
        _prog_cache["nc"] = _build_program()
    nc = _prog_cache["nc"]

    msk = _make_masks()
    in_maps = []
    for c in range(N_CORES):
        bidx, g = c // 4, c % 4
        wk_pad = np.zeros((128, HW), dtype=BF)
        wk_pad[0:LAT] = W_k[:, HW * g : HW * g + HW].astype(BF)
        wv_pad = np.zeros((128, HW), dtype=BF)
        wv_pad[0:LAT] = W_v[:, HW * g : HW * g + HW].astype(BF)
        in_maps.append(
            {
                "xT": np.ascontiguousarray(x[bidx].T).astype(BF),
                "wq": np.ascontiguousarray(W_q[:, HW * g : HW * g + HW]).astype(BF),
                "wkv": W_kv.astype(BF),
                "wk": wk_pad,
                "wv": wv_pad,
                "wo": np.ascontiguousarray(W_o[HW * g : HW * g + HW, :]).astype(BF),
                "msk": msk,
            }
        )

    res = run_bass_kernel_spmd(nc, in_maps, core_ids=list(range(N_CORES)))
    _prog_cache["last_res"] = res

    y = np.zeros((B, T, D), np.float32)
    for c in range(N_CORES):
        y[c // 4] += res.results[c]["y"]
    return y


# revision 38
# speedup vs baseline: 1.0168x; 1.0087x over previous
"""MLA attention kernel for 8 Trainium2 NeuronCores.

Sharding: data-parallel over batch (B=2) x tensor-parallel over heads
(16 heads -> 4 per core). Each core computes full-sequence causal
attention for its 4 heads and a *partial* output projection
(y_part = out_heads @ W_o[head_rows]); the host sums the 4 partials per
batch element.

Dataflow is feature-major ("transposed activations") so no on-chip
transposes are needed anywhere:
  x^T (host-pretransposed, bf16) -> lat^T = W_kv^T x^T -> K^T = W_k^T lat^T
  Q^T = W_q^T x^T,  V = lat^T.T W_v  (token-major, with a ones column
  appended per head so the P@V matmul also yields softmax denominators)
  S^T = K^T.T Q^T per (head, 128-key-block, 512-query-chunk), exp on ACT
  (no max subtraction: |S*scale| <= ~8 for this problem), causal via
  static block skipping + per-block masks on the 4 diagonal blocks,
  out^T accumulated in PSUM over key blocks, normalized by 1/l, then
  y_part^T ... = out^T rows as lhsT for the W_o projection.
"""

import sys

sys.path.insert(0, "/opt/trn_rl_repo")

import numpy as np
import ml_dtypes

BF = ml_dtypes.bfloat16

B, T, D = 2, 4096, 1024
H, DH, LAT = 16, 64, 32
H_LOC = 4              # heads per core
HW = H_LOC * DH        # 256, per-core head width
QC = 512               # query chunk
NQC = T // QC          # 8
KB = 128               # key block
NKB = T // KB          # 32
SCALE = 1.0 / np.sqrt(np.float32(DH))  # 0.125
N_CORES = 8

_prog_cache = {}


def _build_program():
    import concourse.bacc as bacc
    import concourse.tile as tile
    import concourse.mybir as mybir

    f32 = mybir.dt.float32
    bf16 = mybir.dt.bfloat16
    Exp = mybir.ActivationFunctionType.Exp

    nc = bacc.Bacc(None, target_bir_lowering=False)

    xT = nc.dram_tensor("xT", [D, T], bf16, kind="ExternalInput")
    wq = nc.dram_tensor("wq", [D, HW], bf16, kind="ExternalInput")
    wkv = nc.dram_tensor("wkv", [D, LAT], bf16, kind="ExternalInput")
    wk = nc.dram_tensor("wk", [128, HW], bf16, kind="ExternalInput")   # zero-padded 32->128
    wv = nc.dram_tensor("wv", [128, HW], bf16, kind="ExternalInput")   # zero-padded 32->128
    wo = nc.dram_tensor("wo", [HW, D], bf16, kind="ExternalInput")
    msk = nc.dram_tensor("msk", [128, 4, 2 * QC], bf16, kind="ExternalInput")
    y = nc.dram_tensor("y", [T, D], f32, kind="ExternalOutput")

    DSUB = D // 128  # 8

    COPY_ON_ACT = lambda tcidx: tcidx <= 1

    with tile.TileContext(nc) as tc:
        with (
            tc.tile_pool(name="singles", bufs=1) as singles,
            tc.tile_pool(name="work", bufs=6) as work,
            tc.tile_pool(name="pT", bufs=8) as ppool,
            tc.tile_pool(name="ot", bufs=6) as opool,
            tc.tile_pool(name="ysb", bufs=3) as ypool,
            tc.tile_pool(name="lv", bufs=4) as lpool,
            tc.tile_pool(name="ps_s", bufs=2, space="PSUM") as ps_s,
            tc.tile_pool(name="ps_o", bufs=1, space="PSUM") as ps_o,
            tc.tile_pool(name="ps_m", bufs=2, space="PSUM") as ps_m,
        ):
            # ---- persistent SBUF tensors -------------------------------
            # small weights needed by the first chunk go first; the rest are
            # DMA'd after the first x-chunk so chunk 0 isn't starved
            wkv_sb = singles.tile([128, DSUB, LAT], bf16)
            nc.sync.dma_start(wkv_sb, wkv.rearrange("(o p) m -> p o m", p=128))
            wk_sb = singles.tile([128, HW], bf16)
            nc.sync.dma_start(wk_sb, wk[:, :])
            wv_sb = singles.tile([128, HW], bf16)
            nc.sync.dma_start(wv_sb, wv[:, :])
            wq_sb = singles.tile([128, DSUB, HW], bf16)
            wo_sb = singles.tile([128, 2, D], bf16)
            msk_sb = singles.tile([128, 4, 2 * QC], bf16)

            def emit_late_weight_loads():
                nc.sync.dma_start(wq_sb, wq.rearrange("(o p) m -> p o m", p=128))
                nc.sync.dma_start(wo_sb, wo.rearrange("(r p) d -> p r d", p=128))
                nc.sync.dma_start(msk_sb, msk[:, :, :])

            ones_w = DH + 1  # 65
            # per-chunk persistent tiles: breaks write-after-read false deps
            # between chunk tc's projections and chunk tc-1's attention
            kt_c = []
            v1_c = []
            for c in range(NQC):
                kt_1 = singles.tile([128, 2, QC], bf16, name=f"kt_{c}")
                kt_c.append(kt_1)
                v1_1 = singles.tile(
                    [128, QC // KB, H_LOC * ones_w], bf16, name=f"v1_{c}"
                )
                nc.vector.memset(v1_1, 1.0)  # ones columns survive V copies
                v1_c.append(v1_1)

            # warm the ACT exp table-set during the initial weight DMAs so
            # the first real exp doesn't pay the ~2.7us table load
            warm = lpool.tile([1, 1], f32, tag="warm", name="warm")
            nc.scalar.activation(warm, v1_c[0][0:1, 0:1, 0:1], Exp)

            qt_tiles = {}

            xt_tiles = {}

            def emit_xt_dma(tcidx):
                t0 = tcidx * QC
                xt = work.tile([128, DSUB, QC], bf16, tag="xt", name="xt")
                xTv = xT.rearrange("(o p) t -> p o t", p=128)
                nc.sync.dma_start(xt[:, 0:2], xTv[:, 0:2, t0 : t0 + QC])
                nc.sync.dma_start(xt[:, 2:DSUB], xTv[:, 2:DSUB, t0 : t0 + QC])
                xt_tiles[tcidx] = xt

            def p0_copy(tcidx, out, in_):
                if COPY_ON_ACT(tcidx):
                    nc.scalar.copy(out, in_)
                else:
                    nc.vector.tensor_copy(out, in_)

            def emit_phase0_a(tcidx):
                if tcidx not in xt_tiles:
                    emit_xt_dma(tcidx)
                xt = xt_tiles.pop(tcidx)
                # lat^T chunk (local tile; rows 32.. zeroed for the padded
                # contractions in the K/V projections)
                latT = work.tile([128, QC], bf16, tag="latT", name="latT")
                nc.vector.memset(latT, 0.0)
                lt_ps = ps_m.tile([128, QC], f32, tag="m", name="lt_ps")
                for o in range(DSUB):
                    nc.tensor.matmul(
                        lt_ps[0:LAT, 0:QC],
                        lhsT=wkv_sb[:, o],
                        rhs=xt[:, o],
                        start=(o == 0),
                        stop=(o == DSUB - 1),
                    )
                p0_copy(tcidx, latT[0:LAT, :], lt_ps[0:LAT, 0:QC])
                # K^T chunk (2 head pairs)
                for p in range(2):
                    ktp = ps_m.tile([128, QC], f32, tag="m", name="ktp")
                    nc.tensor.matmul(
                        ktp[:, 0:QC],
                        lhsT=wk_sb[:, 128 * p : 128 * p + 128],
                        rhs=latT,
                        start=True,
                        stop=True,
                    )
                    p0_copy(tcidx, kt_c[tcidx][:, p, :], ktp[:, 0:QC])
                qt_tiles[tcidx] = (xt, latT)

            def emit_phase0_b(tcidx):
                xt, latT = qt_tiles[tcidx]
                # V chunk (4 key blocks of 128)
                for s in range(QC // KB):
                    vps = ps_m.tile([128, QC], f32, tag="m", name="vps")
                    nc.tensor.matmul(
                        vps[:, 0:HW],
                        lhsT=latT[:, s * KB : s * KB + KB],
                        rhs=wv_sb,
                        start=True,
                        stop=True,
                    )
                    p0_copy(
                        tcidx,
                        v1_c[tcidx][:, s].rearrange("p (h e) -> p h e", e=ones_w)[
                            :, :, 0:DH
                        ],
                        vps[:, 0:HW].rearrange("p (h d) -> p h d", d=DH),
                    )
                # Q^T chunk (2 head pairs)
                qt = work.tile([128, 2, QC], bf16, tag="qt", name="qt")
                for p in range(2):
                    qtp = ps_m.tile([128, QC], f32, tag="m", name="qtp")
                    for o in range(DSUB):
                        nc.tensor.matmul(
                            qtp[:, 0:QC],
                            lhsT=wq_sb[:, o, 128 * p : 128 * p + 128],
                            rhs=xt[:, o],
                            start=(o == 0),
                            stop=(o == DSUB - 1),
                        )
                    p0_copy(tcidx, qt[:, p, :], qtp[:, 0:QC])
                qt_tiles[tcidx] = qt

            def emit_phase0(tcidx):
                emit_phase0_a(tcidx)
                emit_phase0_b(tcidx)

            def emit_attention_pair(qc, pair, ots):
                qt = qt_tiles[qc]
                nkb = (qc + 1) * (QC // KB)
                if True:
                    o_ps = []
                    for h01 in range(2):
                        o_ps_h = ps_o.tile(
                            [128, QC], f32, tag=f"o{h01}", name=f"o_ps_{h01}"
                        )
                        o_ps.append(o_ps_h)
                    for kb in range(nkb):
                        # diagonal blocks: queries < j*128 are fully masked;
                        # restrict S/exp/mask/PV to the live column subrange
                        j = kb - (nkb - 4)
                        qlo = j * KB if j > 0 else 0
                        qn = QC - qlo
                        s_ps = ps_s.tile([128, 2 * QC], f32, tag="s", name="s_ps")
                        for h01 in range(2):
                            nc.tensor.matmul(
                                s_ps[:, h01 * QC + qlo : h01 * QC + QC],
                                lhsT=kt_c[kb // 4][
                                    64 * h01 : 64 * h01 + 64,
                                    pair,
                                    (kb % 4) * KB : (kb % 4) * KB + KB,
                                ],
                                rhs=qt[64 * h01 : 64 * h01 + 64, pair, qlo:QC],
                                start=True,
                                stop=True,
                            )
                        pT = ppool.tile([128, 2 * QC], bf16, tag="pT", name="pT")
                        s_v = s_ps.rearrange("p (h q) -> p h q", h=2)
                        p_v = pT.rearrange("p (h q) -> p h q", h=2)
                        nc.scalar.activation(
                            p_v[:, :, qlo:QC],
                            s_v[:, :, qlo:QC],
                            Exp,
                            scale=float(SCALE),
                        )
                        if j >= 0:
                            nc.vector.tensor_mul(
                                p_v[:, :, qlo:QC],
                                p_v[:, :, qlo:QC],
                                msk_sb[:, j].rearrange("p (h q) -> p h q", h=2)[
                                    :, :, qlo:QC
                                ],
                            )
                        for h01 in range(2):
                            h = 2 * pair + h01
                            nc.tensor.matmul(
                                o_ps[h01][0 : ones_w, qlo:QC],
                                lhsT=v1_c[kb // 4][
                                    :, kb % 4, ones_w * h : ones_w * h + ones_w
                                ],
                                rhs=pT[:, h01 * QC + qlo : h01 * QC + QC],
                                start=(kb == 0),
                                stop=(kb == nkb - 1),
                            )
                    # copy accumulators to SBUF right away so the PSUM
                    # banks free for the next pair, then normalize from SBUF
                    ot = opool.tile([128, QC], bf16, tag="ot", name="ot")
                    osb = []
                    for h01 in range(2):
                        osb_h = lpool.tile(
                            [ones_w, QC], f32, tag=f"osb{h01}", name="osb_h"
                        )
                        nc.vector.tensor_copy(osb_h, o_ps[h01][0:ones_w, :])
                        osb.append(osb_h)
                    for h01 in range(2):
                        linv = lpool.tile([1, QC], f32, tag="linv", name="linv")
                        nc.vector.reciprocal(linv, osb[h01][DH : DH + 1, :])
                        lb = lpool.tile([64, QC], f32, tag="lb", name="lb")
                        nc.gpsimd.partition_broadcast(lb, linv)
                        nc.vector.tensor_mul(
                            ot[64 * h01 : 64 * h01 + 64, :],
                            osb[h01][0:DH, :],
                            lb,
                        )
                    ots.append(ot)

            def emit_y(qc, ots):
                qt_tiles.pop(qc, None)
                q0 = qc * QC
                # ---- output projection for this chunk ------------------
                for tt in range(QC // 128):
                    for nh in range(2):
                        y_ps = ps_m.tile([128, QC], f32, tag="m", name="y_ps")
                        for p2 in range(2):
                            nc.tensor.matmul(
                                y_ps,
                                lhsT=ots[p2][:, tt * 128 : tt * 128 + 128],
                                rhs=wo_sb[:, p2, nh * QC : nh * QC + QC],
                                start=(p2 == 0),
                                stop=(p2 == 1),
                            )
                        y_sb = ypool.tile([128, QC], f32, tag="ysb", name="y_sb")
                        nc.vector.tensor_copy(y_sb, y_ps)
                        nc.sync.dma_start(
                            y[q0 + tt * 128 : q0 + tt * 128 + 128,
                              nh * QC : nh * QC + QC],
                            y_sb,
                        )

            # software pipeline: phase0 of chunk tc+1 is emitted between the
            # two attention pairs of chunk tc, so per-engine in-order streams
            # overlap projections with the ACT-bound attention
            emit_phase0_a(0)
            emit_late_weight_loads()
            emit_phase0_b(0)
            ots_all = {}
            emit_xt_dma(1)
            emit_xt_dma(2)
            emit_phase0_a(1)
            for tcidx in range(NQC):
                if tcidx + 3 < NQC:
                    emit_xt_dma(tcidx + 3)
                ots = []
                ots_all[tcidx] = ots
                emit_attention_pair(tcidx, 0, ots)
                if tcidx - 1 in ots_all:
                    emit_y(tcidx - 1, ots_all.pop(tcidx - 1))
                if tcidx + 1 < NQC:
                    emit_phase0_b(tcidx + 1)
                emit_attention_pair(tcidx, 1, ots)
                if tcidx + 2 < NQC:
                    emit_phase0_a(tcidx + 2)
            emit_y(NQC - 1, ots_all.pop(NQC - 1))

    nc.finalize()
    return nc


def _make_masks():
    # msk[r, j, q (+QC dup)] = 1.0 if j*128 + r <= q else 0
    r = np.arange(128)[:, None, None]
    jj = np.arange(4)[None, :, None]
    q = np.arange(QC)[None, None, :]
    m = ((jj * KB + r) <= q).astype(BF)
    return np.concatenate([m, m], axis=2)  # duplicate for the head pair


def kernel(x, W_kv, W_k, W_v, W_q, W_o):
    import os

    # the axon NTFF profiling hook is unavailable in this environment;
    # force the plain execute path
    os.environ["BASS_NEVER_TRACE"] = "1"
    from concourse.bass_utils import run_bass_kernel_spmd

    x = np.asarray(x)
    W_kv, W_k, W_v, W_q, W_o = (
        np.asarray(W_kv),
        np.asarray(W_k),
        np.asarray(W_v),
        np.asarray(W_q),
        np.asarray(W_o),
    )

    if "nc" not in _prog_cache:
        _prog_cache["nc"] = _build_program()
    nc = _prog_cache["nc"]

    msk = _make_masks()
    in_maps = []
    for c in range(N_CORES):
        bidx, g = c // 4, c % 4
        wk_pad = np.zeros((128, HW), dtype=BF)
        wk_pad[0:LAT] = W_k[:, HW * g : HW * g + HW].astype(BF)
        wv_pad = np.zeros((128, HW), dtype=BF)
        wv_pad[0:LAT] = W_v[:, HW * g : HW * g + HW].astype(BF)
        in_maps.append(
            {
                "xT": np.ascontiguousarray(x[bidx].T).astype(BF),
                "wq": np.ascontiguousarray(W_q[:, HW * g : HW * g + HW]).astype(BF),
                "wkv": W_kv.astype(BF),
                "wk": wk_pad,
                "wv": wv_pad,
                "wo": np.ascontiguousarray(W_o[HW * g : HW * g + HW, :]).astype(BF),
                "msk": msk,
            }
        )

    res = run_bass_kernel_spmd(nc, in_maps, core_ids=list(range(N_CORES)))
    _prog_cache["last_res"] = res

    y = np.zeros((B, T, D), np.float32)
    for c in range(N_CORES):
        y[c // 4] += res.results[c]["y"]
    return y
